# revision 67
# baseline (speedup 1.0000x reference)
"""Block-causal attention Trainium2 kernel (8 NeuronCores).

Sharding: core c = b*4 + g handles batch b (of 2) and head-group g (4 of 16
heads). Each core computes the qkv projection, rmsnorm + 2-D RoPE,
block-causal attention and a partial output projection for its 256 channels;
the host sums the 4 per-group partials per batch.

v2 design notes (vs v1): the PE HAM clock gate runs the array at 1.2 GHz
unless it sees ~3.4us of sustained activity, so the kernel is structured to
keep the tensor engine dense end-to-end:
  * rmsnorm sums use one block-diagonal [128,8] weights matmul per component
    (4 matmuls/l-chunk instead of 16).
  * r_q / r_k / softmax-denominator broadcasts across partitions are done
    with tiny ones-matrix matmuls on the PE (weights [4,128]/[2,128]) instead
    of DRAM round-trips through a scratch buffer + gpsimd broadcast DMA.
  * softmax normalization happens per frame-pair right after its attention
    group, and the output projection for frame-pair fp-1 is interleaved into
    frame-pair fp's attention block, so there is no serial tail.
  * the Scalar (Activation) engine runs Exp only during attention (the two
    rsqrt batches happen before the first exp; no activation-table thrash).
  * QPl/KPl shuffle DMAs are issued from the scalar queue (lc 0,1: before any
    exp) and the gpsimd queue (lc 2,3).
Matmuls run in bf16 (the fp32r path lowers to slow 2-pass fp32 on this HW).

On-chip layouts (per core):
  Q^T/K^T: feature-on-partition tiles QR/QI/KR/KI [128, 2048]; row 32*hh+j
    <-> head hh, complex pair j (R = even orig dim 2j, I = odd 2j+1).
  V: v_blk [128, 16, 4, 65]: l-tile lt, head h, 64 features + ones col 64 so
    the softmax denominator falls out of the M=65 PV matmul.
  Scores: S^T [keys=128, q] per (head, frame-pair, ktile); block-causal means
    frame t only attends keys < 256*(t+1) -- no mask tensor anywhere.
  exp() needs no max-subtraction (|scores| <= 8 after rmsnorm).
"""

import os
import numpy as np

import concourse.bass as bass
import concourse.mybir as mybir
import concourse.tile as tile
from concourse import bacc
from concourse.bass_utils import run_bass_kernel_spmd

F32 = mybir.dt.float32
BF16 = mybir.dt.bfloat16
AF = mybir.ActivationFunctionType
MUL = mybir.AluOpType.mult
ADD = mybir.AluOpType.add
SUB = mybir.AluOpType.subtract

B, T, NP, D, H = 2, 8, 256, 1024, 16
L = T * NP            # 2048
HD = 64               # head dim
HPG = 4               # heads per group (4 groups x 2 batches = 8 cores)
CPG = HPG * HD        # 256 channels per group
NDT = D // 128        # 8 d-tiles
NLC = L // 512        # 4 l-chunks
NLT = L // 128        # 16 l-tiles
EPS = 1e-6

_CACHE = {}


def _emit(nc, tc, ctx, xT, wqk, wv, wo, wvec2, emat, bias36, costab, sintab,
          out, dbg=None):
    sing = ctx.enter_context(tc.tile_pool(name="sing", bufs=1))
    xp = ctx.enter_context(tc.tile_pool(name="xp", bufs=16))
    tmp = ctx.enter_context(tc.tile_pool(name="tmp", bufs=2))
    sqp = ctx.enter_context(tc.tile_pool(name="sqp", bufs=4))
    ptp = ctx.enter_context(tc.tile_pool(name="ptp", bufs=8))
    osb = ctx.enter_context(tc.tile_pool(name="osb", bufs=3))
    dnp = ctx.enter_context(tc.tile_pool(name="dnp", bufs=2))
    # PSUM: 2-bank general + 2-bank pv + 4-bank score pool = 8 banks; the
    # 4-deep score ring lets the PE run ~2 key-tiles ahead of the
    # Scalar-engine exp instead of ping-ponging per tile
    pps = ctx.enter_context(tc.tile_pool(name="pps", bufs=2, space="PSUM"))
    pvp = ctx.enter_context(tc.tile_pool(name="pvp", bufs=2, space="PSUM"))
    stp = ctx.enter_context(tc.tile_pool(name="stp", bufs=2, space="PSUM"))

    # ---- persistent SBUF; DMA order matters for startup latency ----
    # sync queue: per-dt wqk slices interleave with lc0's x tiles (emitted in
    # phase_a(0)) so the first matmul starts after ~256KB of DMA, not 2MB
    wqk_sb = sing.tile([128, NDT, 512], BF16)
    # gpsimd queue: small tables, then wv (needed at V(lc0))
    wvec2_sb = sing.tile([128, 4, 36], BF16)
    nc.gpsimd.dma_start(out=wvec2_sb[:], in_=wvec2[:])
    emat_sb = sing.tile([64, 4, 128], BF16)
    nc.gpsimd.dma_start(out=emat_sb[:], in_=emat[:])
    wv_sb = sing.tile([128, NDT, CPG], BF16)
    nc.gpsimd.dma_start(out=wv_sb[:], in_=wv.rearrange("(t p) o -> p t o", p=128))
    wo_sb = sing.tile([128, 2, D], BF16)
    cos_sb = sing.tile([128, L], BF16)
    sin_sb = sing.tile([128, L], BF16)

    qk_sb = [sing.tile([128, L], BF16, name=f"qk{i}") for i in range(4)]
    rope_sb = [sing.tile([128, L], BF16, name=f"rope{i}") for i in range(4)]
    v_blk = sing.tile([128, NLT, HPG, 65], BF16)
    att_sb = [sing.tile([128, L], BF16, name=f"att{i}") for i in range(2)]
    ones_v = sing.tile([128, NLT, HPG, 1], F32)
    nc.vector.memset(ones_v[:], 1.0)
    nc.vector.tensor_copy(v_blk[:, :, :, 64:65], ones_v[:])

    bias36_sb = sing.tile([36, 1], F32)
    nc.gpsimd.dma_start(out=bias36_sb[:], in_=bias36[:])

    # rms sums (rows 0-3: q-head means; rows 32-35: k-head sums -- engine
    # partition offsets must be 32-aligned) and their rsqrt
    rs_sb = sing.tile([36, L], F32)
    rq36_sb = sing.tile([36, L], BF16)
    # denominator staging rows at partitions 0 and 32; the other rows stay
    # 1.0 so the 64-row broadcast matmul contracts finite values
    dn64 = sing.tile([64, 512], F32)
    nc.vector.memset(dn64[:], 1.0)
    # per-lc shuffled, per-head-contiguous q/k tiles; K pre-scaled by 0.125*r_k
    QPl = [[sing.tile([128, 512], BF16, name=f"qp{i}_{c}") for c in range(NLC)]
           for i in range(2)]
    KPl = [[sing.tile([128, 512], BF16, name=f"kp{i}_{c}") for c in range(NLC)]
           for i in range(2)]

    # ---- phase A: projections + rms-sum matmuls, per l-chunk ----
    xts = {}

    def v_proj_unit(lc, ls4):
        xt = xts[lc]
        lt = lc * 4 + ls4
        ps = pps.tile([128, HPG, 64], F32, name="vps", tag="ps")
        for dt in range(NDT):
            nc.tensor.matmul(ps[:], xt[dt][:, ls4 * 128:(ls4 + 1) * 128],
                             wv_sb[:, dt, :], start=(dt == 0),
                             stop=(dt == NDT - 1))
        nc.vector.tensor_copy(v_blk[:, lt, :, 0:64], ps[:])

    def v_proj(lc):
        # V projection: l on partitions, ones col for the denominator.
        # Split out from phase_a so lc 2,3 can run as PE filler inside the
        # scalar-bound attention region (keeps the HAM clock gate open).
        xt = xts[lc]
        for ls4 in range(4):
            lt = lc * 4 + ls4
            ps = pps.tile([128, HPG, 64], F32, name="vps", tag="ps")
            for dt in range(NDT):
                nc.tensor.matmul(ps[:], xt[dt][:, ls4 * 128:(ls4 + 1) * 128],
                                 wv_sb[:, dt, :], start=(dt == 0),
                                 stop=(dt == NDT - 1))
            nc.vector.tensor_copy(v_blk[:, lt, :, 0:64], ps[:])

    def phase_a(lc, defer_v=False):
        ls = slice(lc * 512, (lc + 1) * 512)
        xt = []
        for dt in range(NDT):
            if lc == 0:
                nc.sync.dma_start(out=wqk_sb[:, dt, :],
                                  in_=wqk[dt * 128:(dt + 1) * 128, :])
            x1 = xp.tile([128, 512], BF16, name=f"xt{dt}", tag="xt")
            nc.sync.dma_start(out=x1[:], in_=xT[dt * 128:(dt + 1) * 128, ls])
            xt.append(x1)
        xts[lc] = xt
        sqs = []
        if lc == 0:
            # dt-outer into 4 concurrent bank-aligned accumulators: each
            # matmul issues as soon as its own wqk/x d-tile DMA lands, so
            # the first l-chunk streams at DMA pace instead of waiting for
            # all 8 d-tiles per output
            pair = [stp.tile([128, 1024], F32, name=f"qk2{j}", tag="st")
                    for j in range(2)]
            for dt in range(NDT):
                for ot in range(4):
                    nc.tensor.matmul(
                        pair[ot // 2][:, (ot % 2) * 512:(ot % 2 + 1) * 512],
                        wqk_sb[:, dt, ot * 128:(ot + 1) * 128], xt[dt][:],
                        start=(dt == 0), stop=(dt == NDT - 1),
                        skip_group_check=True)
            for ot in range(4):
                src = pair[ot // 2][:, (ot % 2) * 512:(ot % 2 + 1) * 512]
                nc.scalar.activation(qk_sb[ot][:, ls], src, AF.Copy)
                sq = sqp.tile([128, 512], BF16, tag="sq")
                nc.vector.tensor_tensor(sq[:], qk_sb[ot][:, ls],
                                        qk_sb[ot][:, ls], MUL)
                sqs.append(sq)
        else:
            for ot in range(4):             # QR, QI, KR, KI
                ps = stp.tile([128, 512], F32, name="qkps", tag="st")
                for dt in range(NDT):
                    nc.tensor.matmul(ps[:],
                                     wqk_sb[:, dt, ot * 128:(ot + 1) * 128],
                                     xt[dt][:], start=(dt == 0),
                                     stop=(dt == NDT - 1))
                nc.scalar.activation(qk_sb[ot][:, ls], ps[:], AF.Copy)
                sq = sqp.tile([128, 512], BF16, tag="sq")
                nc.vector.tensor_tensor(sq[:], qk_sb[ot][:, ls],
                                        qk_sb[ot][:, ls], MUL)
                sqs.append(sq)
        # block-diagonal rms sums: rs[36, 512] accumulates all 4 components
        rs = pvp.tile([36, 512], F32, name="rs", tag="pv")
        for ot in range(4):
            nc.tensor.matmul(rs[:], wvec2_sb[:, ot, :], sqs[ot][:],
                             start=(ot == 0), stop=(ot == 3),
                             skip_group_check=True)
        nc.vector.tensor_copy(rs_sb[:, ls], rs[:])
        if not defer_v:
            v_proj(lc)

    # ---- rsqrt batch for a pair of l-chunks (Scalar sqrt + DVE recip) ----
    def rsqrt_batch(lcpair):
        bs = slice(lcpair * 1024, (lcpair + 1) * 1024)
        nc.scalar.activation(rs_sb[:, bs], rs_sb[:, bs], AF.Sqrt,
                             bias=bias36_sb[:])
        nc.vector.reciprocal_approx_fast(out=rs_sb[:, bs], in_=rs_sb[:, bs])
        nc.vector.tensor_copy(rq36_sb[:, bs], rs_sb[:, bs])

    # ---- phase B: PE broadcast of r, rope, shuffle, k-scale, per l-chunk --
    def phase_b(lc, eng=None):
        eng = eng or nc.vector
        ls = slice(lc * 512, (lc + 1) * 512)
        # broadcast r_q across each head's 32 pair-rows (PE ones-matmul)
        rqf = pps.tile([128, 512], F32, name="rqf", tag="ps")
        nc.tensor.matmul(rqf[:], emat_sb[0:36, 0, :], rq36_sb[0:36, ls],
                         start=True, stop=True, skip_group_check=True)
        rkf = pps.tile([128, 512], F32, name="rkf", tag="ps")
        nc.tensor.matmul(rkf[:], emat_sb[0:36, 1, :], rq36_sb[0:36, ls],
                         start=True, stop=True, skip_group_check=True)
        rqf_ap, rkf_ap = rqf[:], rkf[:]
        if eng is nc.gpsimd:
            # Pool has no PSUM port: stage the r broadcasts through SBUF
            rqs = tmp.tile([128, 512], F32, tag="rqs")
            nc.vector.tensor_copy(rqs[:], rqf[:])
            rks = tmp.tile([128, 512], F32, tag="rks")
            nc.vector.tensor_copy(rks[:], rkf[:])
            rqf_ap, rkf_ap = rqs[:], rks[:]
        # K pre-scale by 0.125*r_k BEFORE rope (rotation commutes with the
        # per-token scale), so exp needs no per-partition scale and both
        # heads' scores can share one activation op
        for t in (2, 3):
            eng.tensor_tensor(qk_sb[t][:, ls], qk_sb[t][:, ls],
                              rkf_ap, MUL)
        # rope (+ r_q fold on the q side)
        for base in (0, 2):
            xr, xi = qk_sb[base][:, ls], qk_sb[base + 1][:, ls]
            for comp in range(2):
                t1 = tmp.tile([128, 512], F32, tag="t1")
                t2 = tmp.tile([128, 512], F32, tag="t2")
                ca, cb = (cos_sb, sin_sb) if comp == 0 else (sin_sb, cos_sb)
                eng.tensor_tensor(t1[:], xr, ca[:, ls], MUL)
                eng.tensor_tensor(t2[:], xi, cb[:, ls], MUL)
                op = SUB if comp == 0 else ADD
                dst = rope_sb[base + comp][:, ls]
                if base == 0:
                    t3 = tmp.tile([128, 512], F32, tag="t3")
                    eng.tensor_tensor(t3[:], t1[:], t2[:], op)
                    eng.tensor_tensor(dst, t3[:], rqf_ap, MUL)
                else:
                    eng.tensor_tensor(dst, t1[:], t2[:], op)

        # shuffle into per-head contiguous tiles (hwdge queues; scalar queue
        # is free of exp work until attention starts)
        dma_eng = nc.scalar if lc < 2 else nc.gpsimd
        for hp2 in range(2):
            for i2 in range(2):
                h2 = hp2 * 2 + i2
                dma_eng.dma_start(out=QPl[hp2][lc][64 * i2:64 * i2 + 32, :],
                                  in_=rope_sb[0][32 * h2:32 * h2 + 32, ls])
                dma_eng.dma_start(out=QPl[hp2][lc][64 * i2 + 32:64 * i2 + 64, :],
                                  in_=rope_sb[1][32 * h2:32 * h2 + 32, ls])
                dma_eng.dma_start(out=KPl[hp2][lc][64 * i2:64 * i2 + 32, :],
                                  in_=rope_sb[2][32 * h2:32 * h2 + 32, ls])
                dma_eng.dma_start(out=KPl[hp2][lc][64 * i2 + 32:64 * i2 + 64, :],
                                  in_=rope_sb[3][32 * h2:32 * h2 + 32, ls])

    # ---- attention group (frame pair, kt-major) + deferred normalize ----
    # The normalize of group N is emitted inside group N+1 at kt==1 (prio)
    # so its cross-engine chain overlaps N+1's first scores instead of
    # blocking them in the PE FIFO. fillq holds PE filler units (deferred V
    # projections, out-projection l-tiles) popped at kt granularity -- the
    # attention region is Scalar-bound, and fillers placed BETWEEN groups
    # would starve the exp stream.
    fillq = []

    def att_group(hp, fp, prev_tail=None, prio=None):
        nkt_sh, nkt_all = 4 * fp + 2, 4 * fp + 4
        pvps = [pvp.tile([65, 512], F32, name=f"pv{hp}_{fp}_{i}", tag="pv")
                for i in range(2)]
        pend = []

        def flush_pv():
            for kt_, i_, pt_ in pend:
                h_ = hp * 2 + i_
                if kt_ < nkt_sh:
                    nc.tensor.matmul(pvps[i_][:, :], v_blk[:, kt_, h_, :],
                                     pt_[:, 0:512], start=(kt_ == 0),
                                     stop=(kt_ == nkt_all - 1),
                                     skip_group_check=True)
                else:
                    nc.tensor.matmul(pvps[i_][:, 256:512], v_blk[:, kt_, h_, :],
                                     pt_[:, 0:256], start=False,
                                     stop=(kt_ == nkt_all - 1),
                                     skip_group_check=True)
            pend.clear()

        for kt in range(nkt_all):
            lck, kof = kt // 4, (kt % 4) * 128
            ksl = slice(kof, kof + 128)
            shared = kt < nkt_sh
            if shared:
                # both heads' 512-col scores in one 2-bank tile, matmul dsts
                # bank-aligned (non-aligned PSUM matmul dsts crash the HW);
                # one exp covers both -- ACT has a 352-cycle fixed cost/op
                st = stp.tile([128, 1024], F32, name="st2", tag="st")
                for i in range(2):
                    nc.tensor.matmul(st[:, i * 512:(i + 1) * 512],
                                     KPl[hp][lck][64 * i:64 * i + 64, ksl],
                                     QPl[hp][fp][64 * i:64 * i + 64, :],
                                     start=True, stop=True,
                                     skip_group_check=True)
                pt = ptp.tile([128, 1024], BF16, name="pt2", tag="pt")
                nc.scalar.activation(pt[:], st[:], AF.Exp)
                cur = [(kt, i, pt[:, i * 512:(i + 1) * 512]) for i in range(2)]
            else:
                cur = []
                for i in range(2):
                    st = pps.tile([128, 256], F32, name=f"stu{i}", tag="ps")
                    nc.tensor.matmul(st[:],
                                     KPl[hp][lck][64 * i:64 * i + 64, ksl],
                                     QPl[hp][fp][64 * i:64 * i + 64, 256:512],
                                     start=True, stop=True,
                                     skip_group_check=True)
                    pt = ptp.tile([128, 256], BF16, name=f"ptu{i}", tag="pt")
                    nc.scalar.activation(pt[:], st[:], AF.Exp)
                    cur.append((kt, i, pt[:]))
            if len(pend) >= 4:
                flush_pv()
            pend.extend(cur)
            if kt == 0 and prev_tail is not None:
                prev_tail()
            if kt == 1 and prio is not None:
                prio()
            if kt % 2 == 1 and fillq:
                fillq.pop(0)()
                if len(fillq) > 3:
                    fillq.pop(0)()

        def tail():
            flush_pv()

        def norm():
            # normalize: reciprocal of the per-query denominator,
            # PE-broadcast across the 64 feature rows, multiply into att_sb
            qs = slice(fp * 512, fp * 512 + 512)
            nc.vector.tensor_copy(dn64[0:1, :], pvps[0][64:65, :])
            nc.vector.tensor_copy(dn64[32:33, :], pvps[1][64:65, :])
            nc.vector.reciprocal_approx_fast(out=dn64[:], in_=dn64[:])
            dni = dnp.tile([64, 512], BF16, tag="dni")
            nc.vector.tensor_copy(dni[:], dn64[:])
            rinv_ps = pps.tile([128, 512], F32, name="rinv", tag="ps")
            nc.tensor.matmul(rinv_ps[:], emat_sb[:, 3, :], dni[:],
                             start=True, stop=True, skip_group_check=True)
            rinv = tmp.tile([128, 512], F32, tag="rinv_sb")
            nc.vector.tensor_copy(rinv[:], rinv_ps[:])
            for i in range(2):
                nc.vector.tensor_tensor(att_sb[hp][64 * i:64 * i + 64, qs],
                                        pvps[i][0:64, :],
                                        rinv[64 * i:64 * i + 64, :], MUL)
        return tail, norm

    # ---- output projection for one frame pair (4 l-tiles) ----
    def out_proj(fp, ls4s=range(4)):
        for ls4 in ls4s:
            lt = fp * 4 + ls4
            lsl = slice(lt * 128, (lt + 1) * 128)
            pso = [pps.tile([128, 512], F32, name=f"op{oc}", tag="ps")
                   for oc in range(2)]
            for ct in (1, 0):
                for oc in range(2):
                    nc.tensor.matmul(pso[oc][:], att_sb[ct][:, lsl],
                                     wo_sb[:, ct, oc * 512:(oc + 1) * 512],
                                     start=(ct == 1), stop=(ct == 0))
            for oc in range(2):
                ob = osb.tile([128, 512], BF16, tag="ob")
                if (ls4 + oc) % 2 == 0:
                    nc.vector.tensor_copy(ob[:], pso[oc][:])
                else:
                    nc.scalar.activation(ob[:], pso[oc][:], AF.Copy)
                nc.sync.dma_start(out=out[lsl, oc * 512:(oc + 1) * 512],
                                  in_=ob[:])

    # ---- schedule. The attention region is Scalar(exp)-bound, so PE filler
    # (deferred V projections, out-projection halves, B3) is spread between
    # attention groups to keep the tensor engine dense and the HAM clock
    # gate open. fp3 runs hp=1 first so the last normalize finishes during
    # att(0,3) and out_proj(3) starts without stalling. ----
    phase_a(0)
    # rope tables on the scalar queue, needed first by B0's rope (~60us in);
    # emitted late so their DMAs don't hold shared completion semaphores
    # while lc0's x tiles load
    nc.scalar.dma_start(out=cos_sb[:], in_=costab[:])
    nc.scalar.dma_start(out=sin_sb[:], in_=sintab[:])
    phase_a(1)
    nc.gpsimd.dma_start(out=wo_sb[:], in_=wo.rearrange("(t p) o -> p t o", p=128))
    rsqrt_batch(0)
    phase_a(2, defer_v=True)
    phase_b(0)
    phase_a(3, defer_v=True)
    rsqrt_batch(1)
    phase_b(1)
    v_proj(2)
    phase_b(2)
    fillq.extend([lambda i=i: v_proj_unit(3, i) for i in range(4)])
    t, n = att_group(0, 0)
    phase_b(3, eng=nc.gpsimd)
    t, n = att_group(1, 0, prev_tail=t, prio=n)
    fillq.extend([lambda i=i: out_proj(0, [i]) for i in range(4)])
    t, n = att_group(0, 1, prev_tail=t, prio=n)
    t, n = att_group(1, 1, prev_tail=t, prio=n)
    fillq.extend([lambda i=i: out_proj(1, [i]) for i in range(4)])
    t, n = att_group(0, 2, prev_tail=t, prio=n)
    t, n = att_group(1, 2, prev_tail=t, prio=n)
    fillq.extend([lambda i=i: out_proj(2, [i]) for i in range(4)])
    t, n = att_group(1, 3, prev_tail=t, prio=n)
    t, n = att_group(0, 3, prev_tail=t, prio=n)
    t()
    n()
    while fillq:
        fillq.pop(0)()
    out_proj(3)

    if dbg is not None:
        nc.sync.dma_start(out=dbg["rk"], in_=rk_col[:])
        nc.sync.dma_start(out=dbg["rq36"], in_=rq36_sb[:])
        nc.sync.dma_start(out=dbg["att0"], in_=att_sb[0][:])
        nc.sync.dma_start(out=dbg["att1"], in_=att_sb[1][:])
        nc.sync.dma_start(out=dbg["dn"], in_=dn64[:])


def _build_nc():
    import contextlib
    nc = bacc.Bacc("TRN2", target_bir_lowering=False, debug=False, num_devices=8)
    xT = nc.dram_tensor("xT", (D, L), BF16, kind="ExternalInput")
    wqk = nc.dram_tensor("wqk", (D, 512), BF16, kind="ExternalInput")
    wv = nc.dram_tensor("wv", (D, CPG), BF16, kind="ExternalInput")
    wo = nc.dram_tensor("wo", (CPG, D), BF16, kind="ExternalInput")
    wvec2 = nc.dram_tensor("wvec2", (128, 4, 36), BF16, kind="ExternalInput")
    emat = nc.dram_tensor("emat", (64, 4, 128), BF16, kind="ExternalInput")
    bias36 = nc.dram_tensor("bias36", (36, 1), F32, kind="ExternalInput")
    costab = nc.dram_tensor("costab", (128, L), BF16, kind="ExternalInput")
    sintab = nc.dram_tensor("sintab", (128, L), BF16, kind="ExternalInput")
    out = nc.dram_tensor("out", (L, D), BF16, kind="ExternalOutput")
    dbg = None
    if os.environ.get("KDBG"):
        dbg = {
            "rk": nc.dram_tensor("dbg_rk", (128, NLT, 4), F32,
                                 kind="ExternalOutput").ap(),
            "rq36": nc.dram_tensor("dbg_rq36", (36, L), BF16,
                                   kind="ExternalOutput").ap(),
            "att0": nc.dram_tensor("dbg_att0", (128, L), BF16,
                                   kind="ExternalOutput").ap(),
            "att1": nc.dram_tensor("dbg_att1", (128, L), BF16,
                                   kind="ExternalOutput").ap(),
            "dn": nc.dram_tensor("dbg_dn", (64, 512), F32,
                                 kind="ExternalOutput").ap(),
        }

    with tile.TileContext(nc) as tc, contextlib.ExitStack() as ctx:
        _emit(nc, tc, ctx, xT.ap(), wqk.ap(), wv.ap(), wo.ap(), wvec2.ap(),
              emat.ap(), bias36.ap(), costab.ap(), sintab.ap(), out.ap(), dbg)
    nc.compile()
    return nc


def _host_prep(x, Wqkv, Wout, q_scale, k_scale):
    x = np.asarray(x, np.float32)
    Wqkv = np.asarray(Wqkv, np.float32)
    Wout = np.asarray(Wout, np.float32)
    q_scale = np.asarray(q_scale, np.float32)
    k_scale = np.asarray(k_scale, np.float32)

    quarter = HD // 4  # 16
    inv = 1.0 / (10000.0 ** (np.arange(quarter, dtype=np.float64) / quarter))
    tt = np.repeat(np.arange(T), NP).astype(np.float64)
    pp = np.tile(np.arange(NP), T).astype(np.float64)
    ang = np.concatenate([tt[:, None] * inv[None, :], pp[:, None] * inv[None, :]],
                         axis=1)  # (L, 32)

    import ml_dtypes
    costab = np.tile(np.cos(ang).astype(np.float32).T, (4, 1)).astype(
        ml_dtypes.bfloat16)  # (128, L)
    sintab = np.tile(np.sin(ang).astype(np.float32).T, (4, 1)).astype(
        ml_dtypes.bfloat16)

    ev, od = np.arange(0, HD, 2), np.arange(1, HD, 2)
    # block-diagonal rms weights: [128, ot(QR,QI,KR,KI), 36]; k sums sit at
    # output rows 32-35 so both halves are 32-aligned partition reads
    wvec2 = np.zeros((128, 4, 36), np.float32)
    for hh in range(HPG):
        r = slice(32 * hh, 32 * hh + 32)
        wvec2[r, 0, hh] = 1.0 / (HD * q_scale[ev] ** 2)
        wvec2[r, 1, hh] = 1.0 / (HD * q_scale[od] ** 2)
        wvec2[r, 2, 32 + hh] = 1.0 / (k_scale[ev] ** 2)
        wvec2[r, 3, 32 + hh] = 1.0 / (k_scale[od] ** 2)

    # ones matrices for PE partition-broadcast matmuls (zero-padded rows so
    # every rhs can start at a 32-aligned partition)
    emat = np.zeros((64, 4, 128), np.float32)
    for hh in range(HPG):
        emat[hh, 0, 32 * hh:32 * hh + 32] = 1.0    # r_q: head hh -> 32 rows
        emat[32 + hh, 1, 32 * hh:32 * hh + 32] = 1.0   # 0.125*r_k, same rows
    emat[0, 3, 0:64] = 1.0                         # denominator head-pair 0
    emat[32, 3, 64:128] = 1.0                      # denominator head-pair 1

    bias36 = np.full((36, 1), EPS, np.float32)
    bias36[32:36] = 64.0 * EPS

    in_maps = []
    for c in range(8):
        b, g = c // 4, c % 4
        wqk = np.empty((D, 512), np.float32)
        for hh in range(HPG):
            gh = g * HPG + hh
            wq = Wqkv[gh * HD:(gh + 1) * HD, :] * q_scale[:, None]
            wk = Wqkv[D + gh * HD:D + (gh + 1) * HD, :] * k_scale[:, None]
            wqk[:, 0 + 32 * hh:32 + 32 * hh] = wq[ev].T
            wqk[:, 128 + 32 * hh:160 + 32 * hh] = wq[od].T
            wqk[:, 256 + 32 * hh:288 + 32 * hh] = wk[ev].T
            wqk[:, 384 + 32 * hh:416 + 32 * hh] = wk[od].T
        wv = np.ascontiguousarray(
            Wqkv[2 * D + g * CPG:2 * D + (g + 1) * CPG, :].T).astype(ml_dtypes.bfloat16)
        wo = np.ascontiguousarray(Wout[:, g * CPG:(g + 1) * CPG].T)
        in_maps.append({
            "xT": np.ascontiguousarray(x[b].T).astype(ml_dtypes.bfloat16),
            "wqk": wqk.astype(ml_dtypes.bfloat16), "wv": wv,
            "wo": wo.astype(ml_dtypes.bfloat16),
            "wvec2": wvec2.astype(ml_dtypes.bfloat16),
            "emat": emat.astype(ml_dtypes.bfloat16),
            "bias36": bias36,
            "costab": costab, "sintab": sintab,
        })
    return in_maps


def kernel(x, Wqkv, Wout, q_scale, k_scale, T=None, N_p=None):
    assert int(T) == 8 and int(N_p) == 256
    if "nc" not in _CACHE:
        _CACHE["nc"] = _build_nc()
    nc = _CACHE["nc"]
    in_maps = _host_prep(x, Wqkv, Wout, q_scale, k_scale)
    trace = bool(int(os.environ.get("KERNEL_TRACE", "0")))
    res = run_bass_kernel_spmd(nc, in_maps, core_ids=list(range(8)), trace=trace)
    _CACHE["last_exec_time_ns"] = res.exec_time_ns
    outp = np.zeros((B, L, D), np.float32)
    for c in range(8):
        outp[c // 4] += np.asarray(res.results[c]["out"], np.float32)
    _CACHE["results"] = res.results
    return outp


if __name__ == "__main__":
    rng = np.random.default_rng(0)
    x = rng.standard_normal((B, L, D), dtype=np.float32)
    Wqkv = rng.standard_normal((3 * D, D), dtype=np.float32) * 0.02
    Wout = rng.standard_normal((D, D), dtype=np.float32) * 0.02
    o = kernel(x, Wqkv, Wout, np.ones(HD, np.float32), np.ones(HD, np.float32),
               8, 256)
    print("out", o.shape, o.dtype, float(np.abs(o).mean()))


# revision 69
# speedup vs baseline: 1.0524x; 1.0524x over previous
"""Block-causal attention Trainium2 kernel (8 NeuronCores).

Sharding: core c = b*4 + g handles batch b (of 2) and head-group g (4 of 16
heads). Each core computes the qkv projection, rmsnorm + 2-D RoPE,
block-causal attention and a partial output projection for its 256 channels;
the host sums the 4 per-group partials per batch.

v2 design notes (vs v1): the PE HAM clock gate runs the array at 1.2 GHz
unless it sees ~3.4us of sustained activity, so the kernel is structured to
keep the tensor engine dense end-to-end:
  * rmsnorm sums use one block-diagonal [128,8] weights matmul per component
    (4 matmuls/l-chunk instead of 16).
  * r_q / r_k / softmax-denominator broadcasts across partitions are done
    with tiny ones-matrix matmuls on the PE (weights [4,128]/[2,128]) instead
    of DRAM round-trips through a scratch buffer + gpsimd broadcast DMA.
  * softmax normalization happens per frame-pair right after its attention
    group, and the output projection for frame-pair fp-1 is interleaved into
    frame-pair fp's attention block, so there is no serial tail.
  * the Scalar (Activation) engine runs Exp only during attention (the two
    rsqrt batches happen before the first exp; no activation-table thrash).
  * QPl/KPl shuffle DMAs are issued from the scalar queue (lc 0,1: before any
    exp) and the gpsimd queue (lc 2,3).
Matmuls run in bf16 (the fp32r path lowers to slow 2-pass fp32 on this HW).

On-chip layouts (per core):
  Q^T/K^T: feature-on-partition tiles QR/QI/KR/KI [128, 2048]; row 32*hh+j
    <-> head hh, complex pair j (R = even orig dim 2j, I = odd 2j+1).
  V: v_blk [128, 16, 4, 65]: l-tile lt, head h, 64 features + ones col 64 so
    the softmax denominator falls out of the M=65 PV matmul.
  Scores: S^T [keys=128, q] per (head, frame-pair, ktile); block-causal means
    frame t only attends keys < 256*(t+1) -- no mask tensor anywhere.
  exp() needs no max-subtraction (|scores| <= 8 after rmsnorm).
"""

import os
import numpy as np

import concourse.bass as bass
import concourse.mybir as mybir
import concourse.tile as tile
from concourse import bacc
from concourse.bass_utils import run_bass_kernel_spmd

F32 = mybir.dt.float32
BF16 = mybir.dt.bfloat16
AF = mybir.ActivationFunctionType
MUL = mybir.AluOpType.mult
ADD = mybir.AluOpType.add
SUB = mybir.AluOpType.subtract

B, T, NP, D, H = 2, 8, 256, 1024, 16
L = T * NP            # 2048
HD = 64               # head dim
HPG = 4               # heads per group (4 groups x 2 batches = 8 cores)
CPG = HPG * HD        # 256 channels per group
NDT = D // 128        # 8 d-tiles
NLC = L // 512        # 4 l-chunks
NLT = L // 128        # 16 l-tiles
EPS = 1e-6

_CACHE = {}


def _emit(nc, tc, ctx, xT, wqk, wv, wo, wvec2, emat, bias36, costab, sintab,
          out, dbg=None):
    sing = ctx.enter_context(tc.tile_pool(name="sing", bufs=1))
    xp = ctx.enter_context(tc.tile_pool(name="xp", bufs=16))
    tmp = ctx.enter_context(tc.tile_pool(name="tmp", bufs=2))
    sqp = ctx.enter_context(tc.tile_pool(name="sqp", bufs=4))
    ptp = ctx.enter_context(tc.tile_pool(name="ptp", bufs=8))
    osb = ctx.enter_context(tc.tile_pool(name="osb", bufs=3))
    dnp = ctx.enter_context(tc.tile_pool(name="dnp", bufs=2))
    # PSUM: 2-bank general + 2-bank pv + 4-bank score pool = 8 banks; the
    # 4-deep score ring lets the PE run ~2 key-tiles ahead of the
    # Scalar-engine exp instead of ping-ponging per tile
    pps = ctx.enter_context(tc.tile_pool(name="pps", bufs=2, space="PSUM"))
    pvp = ctx.enter_context(tc.tile_pool(name="pvp", bufs=2, space="PSUM"))
    stp = ctx.enter_context(tc.tile_pool(name="stp", bufs=2, space="PSUM"))

    # ---- persistent SBUF; DMA order matters for startup latency ----
    # sync queue: per-dt wqk slices interleave with lc0's x tiles (emitted in
    # phase_a(0)) so the first matmul starts after ~256KB of DMA, not 2MB
    wqk_sb = sing.tile([128, NDT, 512], BF16)
    # gpsimd queue: small tables, then wv (needed at V(lc0))
    wvec2_sb = sing.tile([128, 4, 36], BF16)
    nc.gpsimd.dma_start(out=wvec2_sb[:], in_=wvec2[:])
    emat_sb = sing.tile([64, 4, 128], BF16)
    nc.gpsimd.dma_start(out=emat_sb[:], in_=emat[:])
    wv_sb = sing.tile([128, NDT, CPG], BF16)
    nc.gpsimd.dma_start(out=wv_sb[:], in_=wv.rearrange("(t p) o -> p t o", p=128))
    wo_sb = sing.tile([128, 2, D], BF16)
    cos_sb = sing.tile([128, L], BF16)
    sin_sb = sing.tile([128, L], BF16)

    qk_sb = [sing.tile([128, L], BF16, name=f"qk{i}") for i in range(4)]
    rope_sb = [sing.tile([128, L], BF16, name=f"rope{i}") for i in range(4)]
    v_blk = sing.tile([128, NLT, HPG, 65], BF16)
    att_sb = [sing.tile([128, L], BF16, name=f"att{i}") for i in range(2)]
    ones_v = sing.tile([128, NLT, HPG, 1], F32)
    nc.vector.memset(ones_v[:], 1.0)
    nc.vector.tensor_copy(v_blk[:, :, :, 64:65], ones_v[:])

    bias36_sb = sing.tile([36, 1], F32)
    nc.gpsimd.dma_start(out=bias36_sb[:], in_=bias36[:])

    # rms sums (rows 0-3: q-head means; rows 32-35: k-head sums -- engine
    # partition offsets must be 32-aligned) and their rsqrt
    rs_sb = sing.tile([36, L], F32)
    rq36_sb = sing.tile([36, L], BF16)
    # denominator staging rows at partitions 0 and 32; the other rows stay
    # 1.0 so the 64-row broadcast matmul contracts finite values
    dn64 = sing.tile([64, 512], F32)
    nc.vector.memset(dn64[:], 1.0)
    # per-lc shuffled, per-head-contiguous q/k tiles; K pre-scaled by 0.125*r_k
    QPl = [[sing.tile([128, 512], BF16, name=f"qp{i}_{c}") for c in range(NLC)]
           for i in range(2)]
    KPl = [[sing.tile([128, 512], BF16, name=f"kp{i}_{c}") for c in range(NLC)]
           for i in range(2)]

    # ---- phase A: projections + rms-sum matmuls, per l-chunk ----
    xts = {}

    def v_proj_unit(lc, ls4):
        xt = xts[lc]
        lt = lc * 4 + ls4
        ps = pps.tile([128, HPG, 64], F32, name="vps", tag="ps")
        for dt in range(NDT):
            nc.tensor.matmul(ps[:], xt[dt][:, ls4 * 128:(ls4 + 1) * 128],
                             wv_sb[:, dt, :], start=(dt == 0),
                             stop=(dt == NDT - 1))
        nc.vector.tensor_copy(v_blk[:, lt, :, 0:64], ps[:])

    def v_proj(lc):
        # V projection: l on partitions, ones col for the denominator.
        # Split out from phase_a so lc 2,3 can run as PE filler inside the
        # scalar-bound attention region (keeps the HAM clock gate open).
        xt = xts[lc]
        for ls4 in range(4):
            lt = lc * 4 + ls4
            ps = pps.tile([128, HPG, 64], F32, name="vps", tag="ps")
            for dt in range(NDT):
                nc.tensor.matmul(ps[:], xt[dt][:, ls4 * 128:(ls4 + 1) * 128],
                                 wv_sb[:, dt, :], start=(dt == 0),
                                 stop=(dt == NDT - 1))
            nc.vector.tensor_copy(v_blk[:, lt, :, 0:64], ps[:])

    def phase_a(lc, defer_v=False):
        ls = slice(lc * 512, (lc + 1) * 512)
        xt = []
        for dt in range(NDT):
            if lc == 0:
                nc.sync.dma_start(out=wqk_sb[:, dt, :],
                                  in_=wqk[dt * 128:(dt + 1) * 128, :])
            x1 = xp.tile([128, 512], BF16, name=f"xt{dt}", tag="xt")
            nc.sync.dma_start(out=x1[:], in_=xT[dt * 128:(dt + 1) * 128, ls])
            xt.append(x1)
        xts[lc] = xt
        sqs = []
        if lc == 0:
            # dt-outer into 4 concurrent bank-aligned accumulators: each
            # matmul issues as soon as its own wqk/x d-tile DMA lands, so
            # the first l-chunk streams at DMA pace instead of waiting for
            # all 8 d-tiles per output
            pair = [stp.tile([128, 1024], F32, name=f"qk2{j}", tag="st")
                    for j in range(2)]
            for dt in range(NDT):
                for ot in range(4):
                    nc.tensor.matmul(
                        pair[ot // 2][:, (ot % 2) * 512:(ot % 2 + 1) * 512],
                        wqk_sb[:, dt, ot * 128:(ot + 1) * 128], xt[dt][:],
                        start=(dt == 0), stop=(dt == NDT - 1),
                        skip_group_check=True)
            for ot in range(4):
                src = pair[ot // 2][:, (ot % 2) * 512:(ot % 2 + 1) * 512]
                nc.scalar.activation(qk_sb[ot][:, ls], src, AF.Copy)
                sq = sqp.tile([128, 512], BF16, tag="sq")
                nc.vector.tensor_tensor(sq[:], qk_sb[ot][:, ls],
                                        qk_sb[ot][:, ls], MUL)
                sqs.append(sq)
        else:
            for ot in range(4):             # QR, QI, KR, KI
                ps = stp.tile([128, 512], F32, name="qkps", tag="st")
                for dt in range(NDT):
                    nc.tensor.matmul(ps[:],
                                     wqk_sb[:, dt, ot * 128:(ot + 1) * 128],
                                     xt[dt][:], start=(dt == 0),
                                     stop=(dt == NDT - 1))
                nc.scalar.activation(qk_sb[ot][:, ls], ps[:], AF.Copy)
                sq = sqp.tile([128, 512], BF16, tag="sq")
                nc.vector.tensor_tensor(sq[:], qk_sb[ot][:, ls],
                                        qk_sb[ot][:, ls], MUL)
                sqs.append(sq)
        # block-diagonal rms sums: rs[36, 512] accumulates all 4 components
        rs = pvp.tile([36, 512], F32, name="rs", tag="pv")
        for ot in range(4):
            nc.tensor.matmul(rs[:], wvec2_sb[:, ot, :], sqs[ot][:],
                             start=(ot == 0), stop=(ot == 3),
                             skip_group_check=True)
        nc.vector.tensor_copy(rs_sb[:, ls], rs[:])
        if not defer_v:
            v_proj(lc)

    # ---- rsqrt batch for a pair of l-chunks (Scalar sqrt + DVE recip) ----
    def rsqrt_batch(lcpair):
        bs = slice(lcpair * 1024, (lcpair + 1) * 1024)
        nc.scalar.activation(rs_sb[:, bs], rs_sb[:, bs], AF.Sqrt,
                             bias=bias36_sb[:])
        nc.vector.reciprocal_approx_fast(out=rs_sb[:, bs], in_=rs_sb[:, bs])
        nc.vector.tensor_copy(rq36_sb[:, bs], rs_sb[:, bs])

    # ---- phase B: PE broadcast of r, rope, shuffle, k-scale, per l-chunk --
    def phase_b(lc, eng=None):
        eng = eng or nc.vector
        ls = slice(lc * 512, (lc + 1) * 512)
        # broadcast r_q across each head's 32 pair-rows (PE ones-matmul)
        rqf = pps.tile([128, 512], F32, name="rqf", tag="ps")
        nc.tensor.matmul(rqf[:], emat_sb[0:36, 0, :], rq36_sb[0:36, ls],
                         start=True, stop=True, skip_group_check=True)
        rkf = pps.tile([128, 512], F32, name="rkf", tag="ps")
        nc.tensor.matmul(rkf[:], emat_sb[0:36, 1, :], rq36_sb[0:36, ls],
                         start=True, stop=True, skip_group_check=True)
        rqf_ap, rkf_ap = rqf[:], rkf[:]
        if eng is nc.gpsimd:
            # Pool has no PSUM port: stage the r broadcasts through SBUF
            rqs = tmp.tile([128, 512], F32, tag="rqs")
            nc.vector.tensor_copy(rqs[:], rqf[:])
            rks = tmp.tile([128, 512], F32, tag="rks")
            nc.vector.tensor_copy(rks[:], rkf[:])
            rqf_ap, rkf_ap = rqs[:], rks[:]
        # K pre-scale by 0.125*r_k BEFORE rope (rotation commutes with the
        # per-token scale), so exp needs no per-partition scale and both
        # heads' scores can share one activation op
        for t in (2, 3):
            eng.tensor_tensor(qk_sb[t][:, ls], qk_sb[t][:, ls],
                              rkf_ap, MUL)
        # rope (+ r_q fold on the q side)
        for base in (0, 2):
            xr, xi = qk_sb[base][:, ls], qk_sb[base + 1][:, ls]
            for comp in range(2):
                t1 = tmp.tile([128, 512], F32, tag="t1")
                t2 = tmp.tile([128, 512], F32, tag="t2")
                ca, cb = (cos_sb, sin_sb) if comp == 0 else (sin_sb, cos_sb)
                eng.tensor_tensor(t1[:], xr, ca[:, ls], MUL)
                eng.tensor_tensor(t2[:], xi, cb[:, ls], MUL)
                op = SUB if comp == 0 else ADD
                dst = rope_sb[base + comp][:, ls]
                if base == 0:
                    t3 = tmp.tile([128, 512], F32, tag="t3")
                    eng.tensor_tensor(t3[:], t1[:], t2[:], op)
                    eng.tensor_tensor(dst, t3[:], rqf_ap, MUL)
                else:
                    eng.tensor_tensor(dst, t1[:], t2[:], op)

        # shuffle into per-head contiguous tiles (hwdge queues; scalar queue
        # is free of exp work until attention starts)
        dma_eng = nc.scalar if lc < 2 else nc.gpsimd
        for hp2 in range(2):
            for i2 in range(2):
                h2 = hp2 * 2 + i2
                dma_eng.dma_start(out=QPl[hp2][lc][64 * i2:64 * i2 + 32, :],
                                  in_=rope_sb[0][32 * h2:32 * h2 + 32, ls])
                dma_eng.dma_start(out=QPl[hp2][lc][64 * i2 + 32:64 * i2 + 64, :],
                                  in_=rope_sb[1][32 * h2:32 * h2 + 32, ls])
                dma_eng.dma_start(out=KPl[hp2][lc][64 * i2:64 * i2 + 32, :],
                                  in_=rope_sb[2][32 * h2:32 * h2 + 32, ls])
                dma_eng.dma_start(out=KPl[hp2][lc][64 * i2 + 32:64 * i2 + 64, :],
                                  in_=rope_sb[3][32 * h2:32 * h2 + 32, ls])

    # ---- attention group (frame pair, kt-major) + deferred normalize ----
    # The normalize of group N is emitted inside group N+1 at kt==1 (prio)
    # so its cross-engine chain overlaps N+1's first scores instead of
    # blocking them in the PE FIFO. fillq holds PE filler units (deferred V
    # projections, out-projection l-tiles) popped at kt granularity -- the
    # attention region is Scalar-bound, and fillers placed BETWEEN groups
    # would starve the exp stream.
    fillq = []

    def att_group(hp, fp, prev_tail=None, prio=None):
        nkt_sh, nkt_all = 4 * fp + 2, 4 * fp + 4
        pvps = [pvp.tile([65, 512], F32, name=f"pv{hp}_{fp}_{i}", tag="pv")
                for i in range(2)]
        pend = []

        def flush_pv():
            for kt_, i_, pt_ in pend:
                h_ = hp * 2 + i_
                if kt_ < nkt_sh:
                    nc.tensor.matmul(pvps[i_][:, :], v_blk[:, kt_, h_, :],
                                     pt_[:, 0:512], start=(kt_ == 0),
                                     stop=(kt_ == nkt_all - 1),
                                     skip_group_check=True)
                else:
                    nc.tensor.matmul(pvps[i_][:, 256:512], v_blk[:, kt_, h_, :],
                                     pt_[:, 0:256], start=False,
                                     stop=(kt_ == nkt_all - 1),
                                     skip_group_check=True)
            pend.clear()

        for kt in range(nkt_all):
            lck, kof = kt // 4, (kt % 4) * 128
            ksl = slice(kof, kof + 128)
            shared = kt < nkt_sh
            if shared:
                # both heads' 512-col scores in one 2-bank tile, matmul dsts
                # bank-aligned (non-aligned PSUM matmul dsts crash the HW);
                # one exp covers both -- ACT has a 352-cycle fixed cost/op
                st = stp.tile([128, 1024], F32, name="st2", tag="st")
                for i in range(2):
                    nc.tensor.matmul(st[:, i * 512:(i + 1) * 512],
                                     KPl[hp][lck][64 * i:64 * i + 64, ksl],
                                     QPl[hp][fp][64 * i:64 * i + 64, :],
                                     start=True, stop=True,
                                     skip_group_check=True)
                pt = ptp.tile([128, 1024], BF16, name="pt2", tag="pt")
                nc.scalar.activation(pt[:], st[:], AF.Exp)
                cur = [(kt, i, pt[:, i * 512:(i + 1) * 512]) for i in range(2)]
            else:
                cur = []
                for i in range(2):
                    st = stp.tile([128, 256], F32, name=f"stu{i}", tag="st")
                    nc.tensor.matmul(st[:],
                                     KPl[hp][lck][64 * i:64 * i + 64, ksl],
                                     QPl[hp][fp][64 * i:64 * i + 64, 256:512],
                                     start=True, stop=True,
                                     skip_group_check=True)
                    pt = ptp.tile([128, 256], BF16, name=f"ptu{i}", tag="pt")
                    nc.scalar.activation(pt[:], st[:], AF.Exp)
                    cur.append((kt, i, pt[:]))
            if len(pend) >= 4:
                flush_pv()
            pend.extend(cur)
            if kt == 0 and prev_tail is not None:
                prev_tail()
            if kt == 1 and prio is not None:
                prio()
            if kt % 2 == 1 and fillq:
                fillq.pop(0)()
                if len(fillq) > 3:
                    fillq.pop(0)()

        def tail():
            flush_pv()

        def norm():
            # normalize: reciprocal of the per-query denominator,
            # PE-broadcast across the 64 feature rows, multiply into att_sb
            qs = slice(fp * 512, fp * 512 + 512)
            nc.vector.tensor_copy(dn64[0:1, :], pvps[0][64:65, :])
            nc.vector.tensor_copy(dn64[32:33, :], pvps[1][64:65, :])
            nc.vector.reciprocal_approx_fast(out=dn64[:], in_=dn64[:])
            dni = dnp.tile([64, 512], BF16, tag="dni")
            nc.vector.tensor_copy(dni[:], dn64[:])
            rinv_ps = pps.tile([128, 512], F32, name="rinv", tag="ps")
            nc.tensor.matmul(rinv_ps[:], emat_sb[:, 3, :], dni[:],
                             start=True, stop=True, skip_group_check=True)
            rinv = tmp.tile([128, 512], F32, tag="rinv_sb")
            nc.vector.tensor_copy(rinv[:], rinv_ps[:])
            for i in range(2):
                nc.vector.tensor_tensor(att_sb[hp][64 * i:64 * i + 64, qs],
                                        pvps[i][0:64, :],
                                        rinv[64 * i:64 * i + 64, :], MUL)
        return tail, norm

    # ---- output projection for one frame pair (4 l-tiles) ----
    def out_proj(fp, ls4s=range(4), dma2=False):
        for ls4 in ls4s:
            lt = fp * 4 + ls4
            lsl = slice(lt * 128, (lt + 1) * 128)
            pso = [pps.tile([128, 512], F32, name=f"op{oc}", tag="ps")
                   for oc in range(2)]
            for ct in range(2):
                for oc in range(2):
                    nc.tensor.matmul(pso[oc][:], att_sb[ct][:, lsl],
                                     wo_sb[:, ct, oc * 512:(oc + 1) * 512],
                                     start=(ct == 0), stop=(ct == 1))
            for oc in range(2):
                ob = osb.tile([128, 512], BF16, tag="ob")
                if (ls4 + oc) % 2 == 0:
                    nc.vector.tensor_copy(ob[:], pso[oc][:])
                else:
                    nc.scalar.activation(ob[:], pso[oc][:], AF.Copy)
                # the final l-tiles drain after the last exp: split their
                # DMAs across two queues (Scalar is idle by then)
                dma_eng = nc.scalar if (dma2 and oc == 1) else nc.sync
                dma_eng.dma_start(out=out[lsl, oc * 512:(oc + 1) * 512],
                                  in_=ob[:])

    # ---- schedule. The attention region is Scalar(exp)-bound, so PE filler
    # (deferred V projections, out-projection halves, B3) is spread between
    # attention groups to keep the tensor engine dense and the HAM clock
    # gate open. fp3 runs hp=1 first so the last normalize finishes during
    # att(0,3) and out_proj(3) starts without stalling. ----
    phase_a(0)
    # rope tables on the scalar queue, needed first by B0's rope (~60us in);
    # emitted late so their DMAs don't hold shared completion semaphores
    # while lc0's x tiles load
    nc.scalar.dma_start(out=cos_sb[:], in_=costab[:])
    nc.scalar.dma_start(out=sin_sb[:], in_=sintab[:])
    phase_a(1)
    nc.gpsimd.dma_start(out=wo_sb[:], in_=wo.rearrange("(t p) o -> p t o", p=128))
    rsqrt_batch(0)
    phase_a(2, defer_v=True)
    phase_b(0)
    phase_a(3, defer_v=True)
    rsqrt_batch(1)
    phase_b(1)
    v_proj(2)
    phase_b(2)
    fillq.extend([lambda i=i: v_proj_unit(3, i) for i in range(4)])
    t, n = att_group(0, 0)
    phase_b(3, eng=nc.gpsimd)
    t, n = att_group(1, 0, prev_tail=t, prio=n)
    fillq.extend([lambda i=i: out_proj(0, [i]) for i in range(4)])
    t, n = att_group(0, 1, prev_tail=t, prio=n)
    t, n = att_group(1, 1, prev_tail=t, prio=n)
    fillq.extend([lambda i=i: out_proj(1, [i]) for i in range(4)])
    t, n = att_group(0, 2, prev_tail=t, prio=n)
    t, n = att_group(1, 2, prev_tail=t, prio=n)
    fillq.extend([lambda i=i: out_proj(2, [i]) for i in range(4)])
    t, n = att_group(1, 3, prev_tail=t, prio=n)
    t, n = att_group(0, 3, prev_tail=t, prio=n)
    t()
    n()
    while fillq:
        fillq.pop(0)()
    out_proj(3, dma2=True)

    if dbg is not None:
        nc.sync.dma_start(out=dbg["rk"], in_=rk_col[:])
        nc.sync.dma_start(out=dbg["rq36"], in_=rq36_sb[:])
        nc.sync.dma_start(out=dbg["att0"], in_=att_sb[0][:])
        nc.sync.dma_start(out=dbg["att1"], in_=att_sb[1][:])
        nc.sync.dma_start(out=dbg["dn"], in_=dn64[:])


def _build_nc():
    import contextlib
    nc = bacc.Bacc("TRN2", target_bir_lowering=False, debug=False, num_devices=8)
    xT = nc.dram_tensor("xT", (D, L), BF16, kind="ExternalInput")
    wqk = nc.dram_tensor("wqk", (D, 512), BF16, kind="ExternalInput")
    wv = nc.dram_tensor("wv", (D, CPG), BF16, kind="ExternalInput")
    wo = nc.dram_tensor("wo", (CPG, D), BF16, kind="ExternalInput")
    wvec2 = nc.dram_tensor("wvec2", (128, 4, 36), BF16, kind="ExternalInput")
    emat = nc.dram_tensor("emat", (64, 4, 128), BF16, kind="ExternalInput")
    bias36 = nc.dram_tensor("bias36", (36, 1), F32, kind="ExternalInput")
    costab = nc.dram_tensor("costab", (128, L), BF16, kind="ExternalInput")
    sintab = nc.dram_tensor("sintab", (128, L), BF16, kind="ExternalInput")
    out = nc.dram_tensor("out", (L, D), BF16, kind="ExternalOutput")
    dbg = None
    if os.environ.get("KDBG"):
        dbg = {
            "rk": nc.dram_tensor("dbg_rk", (128, NLT, 4), F32,
                                 kind="ExternalOutput").ap(),
            "rq36": nc.dram_tensor("dbg_rq36", (36, L), BF16,
                                   kind="ExternalOutput").ap(),
            "att0": nc.dram_tensor("dbg_att0", (128, L), BF16,
                                   kind="ExternalOutput").ap(),
            "att1": nc.dram_tensor("dbg_att1", (128, L), BF16,
                                   kind="ExternalOutput").ap(),
            "dn": nc.dram_tensor("dbg_dn", (64, 512), F32,
                                 kind="ExternalOutput").ap(),
        }

    with tile.TileContext(nc) as tc, contextlib.ExitStack() as ctx:
        _emit(nc, tc, ctx, xT.ap(), wqk.ap(), wv.ap(), wo.ap(), wvec2.ap(),
              emat.ap(), bias36.ap(), costab.ap(), sintab.ap(), out.ap(), dbg)
    nc.compile()
    return nc


def _host_prep(x, Wqkv, Wout, q_scale, k_scale):
    x = np.asarray(x, np.float32)
    Wqkv = np.asarray(Wqkv, np.float32)
    Wout = np.asarray(Wout, np.float32)
    q_scale = np.asarray(q_scale, np.float32)
    k_scale = np.asarray(k_scale, np.float32)

    quarter = HD // 4  # 16
    inv = 1.0 / (10000.0 ** (np.arange(quarter, dtype=np.float64) / quarter))
    tt = np.repeat(np.arange(T), NP).astype(np.float64)
    pp = np.tile(np.arange(NP), T).astype(np.float64)
    ang = np.concatenate([tt[:, None] * inv[None, :], pp[:, None] * inv[None, :]],
                         axis=1)  # (L, 32)

    import ml_dtypes
    costab = np.tile(np.cos(ang).astype(np.float32).T, (4, 1)).astype(
        ml_dtypes.bfloat16)  # (128, L)
    sintab = np.tile(np.sin(ang).astype(np.float32).T, (4, 1)).astype(
        ml_dtypes.bfloat16)

    ev, od = np.arange(0, HD, 2), np.arange(1, HD, 2)
    # block-diagonal rms weights: [128, ot(QR,QI,KR,KI), 36]; k sums sit at
    # output rows 32-35 so both halves are 32-aligned partition reads
    wvec2 = np.zeros((128, 4, 36), np.float32)
    for hh in range(HPG):
        r = slice(32 * hh, 32 * hh + 32)
        wvec2[r, 0, hh] = 1.0 / (HD * q_scale[ev] ** 2)
        wvec2[r, 1, hh] = 1.0 / (HD * q_scale[od] ** 2)
        wvec2[r, 2, 32 + hh] = 1.0 / (k_scale[ev] ** 2)
        wvec2[r, 3, 32 + hh] = 1.0 / (k_scale[od] ** 2)

    # ones matrices for PE partition-broadcast matmuls (zero-padded rows so
    # every rhs can start at a 32-aligned partition)
    emat = np.zeros((64, 4, 128), np.float32)
    for hh in range(HPG):
        emat[hh, 0, 32 * hh:32 * hh + 32] = 1.0    # r_q: head hh -> 32 rows
        emat[32 + hh, 1, 32 * hh:32 * hh + 32] = 1.0   # 0.125*r_k, same rows
    emat[0, 3, 0:64] = 1.0                         # denominator head-pair 0
    emat[32, 3, 64:128] = 1.0                      # denominator head-pair 1

    bias36 = np.full((36, 1), EPS, np.float32)
    bias36[32:36] = 64.0 * EPS

    in_maps = []
    for c in range(8):
        b, g = c // 4, c % 4
        wqk = np.empty((D, 512), np.float32)
        for hh in range(HPG):
            gh = g * HPG + hh
            wq = Wqkv[gh * HD:(gh + 1) * HD, :] * q_scale[:, None]
            wk = Wqkv[D + gh * HD:D + (gh + 1) * HD, :] * k_scale[:, None]
            wqk[:, 0 + 32 * hh:32 + 32 * hh] = wq[ev].T
            wqk[:, 128 + 32 * hh:160 + 32 * hh] = wq[od].T
            wqk[:, 256 + 32 * hh:288 + 32 * hh] = wk[ev].T
            wqk[:, 384 + 32 * hh:416 + 32 * hh] = wk[od].T
        wv = np.ascontiguousarray(
            Wqkv[2 * D + g * CPG:2 * D + (g + 1) * CPG, :].T).astype(ml_dtypes.bfloat16)
        wo = np.ascontiguousarray(Wout[:, g * CPG:(g + 1) * CPG].T)
        in_maps.append({
            "xT": np.ascontiguousarray(x[b].T).astype(ml_dtypes.bfloat16),
            "wqk": wqk.astype(ml_dtypes.bfloat16), "wv": wv,
            "wo": wo.astype(ml_dtypes.bfloat16),
            "wvec2": wvec2.astype(ml_dtypes.bfloat16),
            "emat": emat.astype(ml_dtypes.bfloat16),
            "bias36": bias36,
            "costab": costab, "sintab": sintab,
        })
    return in_maps


def kernel(x, Wqkv, Wout, q_scale, k_scale, T=None, N_p=None):
    assert int(T) == 8 and int(N_p) == 256
    if "nc" not in _CACHE:
        _CACHE["nc"] = _build_nc()
    nc = _CACHE["nc"]
    in_maps = _host_prep(x, Wqkv, Wout, q_scale, k_scale)
    trace = bool(int(os.environ.get("KERNEL_TRACE", "0")))
    res = run_bass_kernel_spmd(nc, in_maps, core_ids=list(range(8)), trace=trace)
    _CACHE["last_exec_time_ns"] = res.exec_time_ns
    outp = np.zeros((B, L, D), np.float32)
    for c in range(8):
        outp[c // 4] += np.asarray(res.results[c]["out"], np.float32)
    _CACHE["results"] = res.results
    return outp


if __name__ == "__main__":
    rng = np.random.default_rng(0)
    x = rng.standard_normal((B, L, D), dtype=np.float32)
    Wqkv = rng.standard_normal((3 * D, D), dtype=np.float32) * 0.02
    Wout = rng.standard_normal((D, D), dtype=np.float32) * 0.02
    o = kernel(x, Wqkv, Wout, np.ones(HD, np.float32), np.ones(HD, np.float32),
               8, 256)
    print("out", o.shape, o.dtype, float(np.abs(o).mean()))
